# revision 2
# baseline (speedup 1.0000x reference)
"""Trainium2 Bass kernel for BSplineLayer: y = BSpline(knots, coeffs, k=3)((x - min(x)) / (max(x) - min(x) + 1e-8)).

Because the reference clips the de Boor interval index to [k, n-1] = [3, 3]
(n = len(knots) - k - 1 = 4 basis functions), the whole layer reduces to a
single cubic polynomial P(xn) evaluated everywhere, with coefficients that
depend only on knots/coeffs.  The device computes the global min/max
(all-gathered across the 8 cores), folds the normalization into the cubic,
and evaluates it with two fused scalar_tensor_tensor passes + one ACT pass.

Per-core layout: x is sharded row-wise (1024 rows/core), kept SBUF-resident
(16 MiB) so HBM traffic is one read + one write of the shard.
"""

import sys

sys.path.insert(0, "/opt/trn_rl_repo")

import numpy as np

N_CORES = 8
ROWS, COLS = 8192, 4096
R_CORE = ROWS // N_CORES          # 1024 rows per core
P = 128                           # SBUF partitions
N_TILES = R_CORE // P             # 8 tiles of [128, 4096] per core
CHUNK = 2048                      # phase-2 free-dim chunk
DEGREE = 3

_CACHE = {}


def _expand_cubic(knots: np.ndarray, coeffs: np.ndarray) -> np.ndarray:
    """Expand de Boor at interval m=3 into monomial coeffs [a0, a1, a2, a3] (float64)."""
    t = np.asarray(knots, dtype=np.float64)
    c = np.asarray(coeffs, dtype=np.float64)
    k = DEGREE
    m = k  # reference clips searchsorted result to [k, n-1] with n-1 == k
    pm = np.polynomial.polynomial
    d = [np.array([c[m - k + j]], dtype=np.float64) for j in range(k + 1)]
    for r in range(1, k + 1):
        for j in range(k, r - 1, -1):
            tl = t[m - k + j]
            tr = t[m + j + 1 - r]
            inv = 1.0 / (tr - tl)
            alpha = np.array([-tl * inv, inv])
            one_m = np.array([1.0 + tl * inv, -inv])
            d[j] = pm.polyadd(pm.polymul(one_m, d[j - 1]), pm.polymul(alpha, d[j]))
    a = np.zeros(4, dtype=np.float64)
    a[: len(d[k])] = d[k]
    return a


def _build_program():
    import concourse.bass as bass
    import concourse.tile as tile
    from concourse import bacc, bass_isa, mybir

    dt = mybir.dt.float32
    OP = mybir.AluOpType
    AX = mybir.AxisListType
    AF = mybir.ActivationFunctionType

    nc = bacc.Bacc("TRN2", target_bir_lowering=False, debug=False, num_devices=N_CORES)
    x_ext = nc.declare_dram_parameter("x", [R_CORE, COLS], dt, isOutput=False)
    ac_ext = nc.declare_dram_parameter("ac", [1, 4], dt, isOutput=False)
    y_ext = nc.declare_dram_parameter("y", [R_CORE, COLS], dt, isOutput=True)

    with tile.TileContext(nc) as tc:
        with (
            tc.tile_pool(name="xp", bufs=1) as xp,
            tc.tile_pool(name="wp", bufs=2) as wp,
            tc.tile_pool(name="small", bufs=1) as small,
            tc.tile_pool(name="dram", bufs=1, space="DRAM") as dram,
        ):
            # ---------------- phase 1: load + local min/max ----------------
            xts = []
            rmax8 = small.tile([P, N_TILES], dt)
            rmin8 = small.tile([P, N_TILES], dt)
            for t in range(N_TILES):
                xt = xp.tile([P, COLS], dt, tag=f"x{t}")
                xts.append(xt)
                nc.sync.dma_start(out=xt[:], in_=x_ext[t * P:(t + 1) * P, :])
                nc.vector.tensor_reduce(rmax8[:, t:t + 1], xt[:], axis=AX.X, op=OP.max)
                nc.vector.tensor_reduce(rmin8[:, t:t + 1], xt[:], axis=AX.X, op=OP.min)

            pk = small.tile([P, 2], dt)
            nc.vector.tensor_reduce(pk[:, 0:1], rmax8[:], axis=AX.X, op=OP.max)
            rmn = small.tile([P, 1], dt)
            nc.vector.tensor_reduce(rmn[:], rmin8[:], axis=AX.X, op=OP.min)
            nc.vector.tensor_scalar_mul(pk[:, 1:2], rmn[:], -1.0)

            # cross-partition: every partition gets (local_max, -local_min)
            par = small.tile([P, 2], dt)
            nc.gpsimd.partition_all_reduce(par[:], pk[:], channels=P,
                                           reduce_op=bass_isa.ReduceOp.max)

            # cross-core: AllGather the pair, reduce on the way back in
            cc_in = dram.tile([1, 2], dt)
            cc_out = dram.tile([1, 2 * N_CORES], dt)
            nc.sync.dma_start(out=cc_in[:], in_=par[0:1, 0:2])
            nc.gpsimd.collective_compute(
                "AllGather", OP.bypass,
                replica_groups=[list(range(N_CORES))],
                ins=[cc_in[:].opt()], outs=[cc_out[:].opt()],
            )
            g = small.tile([1, 2, N_CORES], dt)
            nc.sync.dma_start(out=g[:], in_=cc_out[:].rearrange("p (r j) -> p j r", j=2))
            gg = small.tile([1, 2], dt)
            nc.vector.tensor_reduce(gg[:], g[:], axis=AX.X, op=OP.max)
            GG = small.tile([P, 2], dt)
            nc.gpsimd.partition_broadcast(GG[:], gg[:])

            # host constants in: ac = [e2a=a2/a3, e1a=a1/a3, a3, a0]
            ac_sb = small.tile([1, 4], dt)
            nc.sync.dma_start(out=ac_sb[:], in_=ac_ext[:])
            AC = small.tile([P, 4], dt)
            nc.gpsimd.partition_broadcast(AC[:], ac_sb[:])
            e2a, e1a, a3c, a0c = (AC[:, i:i + 1] for i in range(4))

            # ------- device scalars: normalization + composed coefficients -------
            # s = 1/(gmax + gnm + eps); b = gnm*s    (gnm = -gmin)
            # y = P(s*x + b) = ((x + d2)*x + d1)*x*q3 + q0
            #   d2 = (3b + e2a)*d        (d = 1/s)
            #   d1 = ((3b + 2*e2a)*b + e1a)*d^2
            #   q3 = a3*s^3
            #   q0 = a3*(b + e2a)*b^2 + a3*e1a*b + a0
            cf = small.tile([P, 4], dt)
            d2c, d1c, q3c, q0c = (cf[:, i:i + 1] for i in range(4))
            tmp = small.tile([P, 8], dt)
            dd, s_, b_, u, v, w, s2, p_ = (tmp[:, i:i + 1] for i in range(8))

            nc.vector.scalar_tensor_tensor(dd, GG[:, 0:1], 1e-8, GG[:, 1:2],
                                           op0=OP.add, op1=OP.add)      # d = range+eps
            nc.vector.reciprocal(s_, dd)
            nc.vector.tensor_tensor(b_, GG[:, 1:2], s_, op=OP.mult)     # b = gnm*s

            nc.vector.tensor_scalar_mul(u, b_, 3.0)                     # u = 3b
            nc.vector.tensor_tensor(v, u, e2a, op=OP.add)               # v = 3b+e2a
            nc.vector.tensor_tensor(d2c, v, dd, op=OP.mult)             # d2

            nc.vector.scalar_tensor_tensor(w, e2a, 2.0, u, op0=OP.mult, op1=OP.add)  # w = 2e2a+3b
            nc.vector.tensor_tensor(w, w, b_, op=OP.mult)
            nc.vector.tensor_tensor(w, w, e1a, op=OP.add)               # (3b+2e2a)b+e1a
            nc.vector.tensor_tensor(v, dd, dd, op=OP.mult)              # v = d^2
            nc.vector.tensor_tensor(d1c, w, v, op=OP.mult)              # d1

            nc.vector.tensor_tensor(s2, s_, s_, op=OP.mult)
            nc.vector.tensor_tensor(u, s2, s_, op=OP.mult)              # s^3
            nc.vector.tensor_tensor(q3c, u, a3c, op=OP.mult)            # q3

            nc.vector.tensor_tensor(p_, b_, e2a, op=OP.add)             # b+e2a
            nc.vector.tensor_tensor(p_, p_, b_, op=OP.mult)
            nc.vector.tensor_tensor(p_, p_, e1a, op=OP.add)             # (b+e2a)b+e1a
            nc.vector.tensor_tensor(p_, p_, b_, op=OP.mult)
            nc.vector.tensor_tensor(p_, p_, a3c, op=OP.mult)            # a3*(...)
            nc.vector.tensor_tensor(q0c, p_, a0c, op=OP.add)            # q0

            # ---------------- phase 2: evaluate + store ----------------
            n_chunks = COLS // CHUNK
            for t in range(N_TILES):
                xt = xts[t]
                for h in range(n_chunks):
                    xc = xt[:, h * CHUNK:(h + 1) * CHUNK]
                    w1 = wp.tile([P, CHUNK], dt, tag="w1")
                    w2 = wp.tile([P, CHUNK], dt, tag="w2")
                    nc.vector.scalar_tensor_tensor(w1[:], xc, d2c, xc,
                                                   op0=OP.add, op1=OP.mult)
                    nc.vector.scalar_tensor_tensor(w2[:], w1[:], d1c, xc,
                                                   op0=OP.add, op1=OP.mult)
                    nc.scalar.activation(xc, w2[:], AF.Identity,
                                         bias=q0c, scale=q3c)
                nc.sync.dma_start(out=y_ext[t * P:(t + 1) * P, :], in_=xt[:])

    nc.compile()
    return nc


def kernel(x: np.ndarray, knots: np.ndarray, coeffs: np.ndarray) -> np.ndarray:
    from concourse.bass_utils import run_bass_kernel_spmd

    x = np.ascontiguousarray(np.asarray(x, dtype=np.float32))
    assert x.shape == (ROWS, COLS), x.shape

    a = _expand_cubic(knots, coeffs)
    a3 = a[3] if abs(a[3]) > 1e-30 else 1e-30
    ac = np.array([[a[2] / a3, a[1] / a3, a3, a[0]]], dtype=np.float32)

    if "nc" not in _CACHE:
        _CACHE["nc"] = _build_program()
    nc = _CACHE["nc"]

    shards = [x[i * R_CORE:(i + 1) * R_CORE] for i in range(N_CORES)]
    in_maps = [{"x": s, "ac": ac} for s in shards]

    import os
    trace = bool(int(os.environ.get("KERNEL_TRACE", "0")))
    res = run_bass_kernel_spmd(nc, in_maps, core_ids=list(range(N_CORES)),
                               trace=trace)
    if trace and res.exec_time_ns is not None:
        print(f"HW exec time: {res.exec_time_ns} ns")
        _CACHE["last_exec_time_ns"] = res.exec_time_ns
        _CACHE["last_trace"] = res.instructions_and_trace

    out = np.empty((ROWS, COLS), dtype=np.float32)
    for i in range(N_CORES):
        out[i * R_CORE:(i + 1) * R_CORE] = res.results[i]["y"]
    return out


# revision 3
# speedup vs baseline: 1.0199x; 1.0199x over previous
"""Trainium2 Bass kernel for BSplineLayer: y = BSpline(knots, coeffs, k=3)((x - min(x)) / (max(x) - min(x) + 1e-8)).

Because the reference clips the de Boor interval index to [k, n-1] = [3, 3]
(n = len(knots) - k - 1 = 4 basis functions), the whole layer reduces to a
single cubic polynomial P(xn) evaluated everywhere, with coefficients that
depend only on knots/coeffs.  The device computes the global min/max
(all-gathered across the 8 cores), folds the normalization into the cubic,
and evaluates it with two fused scalar_tensor_tensor passes + one ACT pass.

Per-core layout: x is sharded row-wise (1024 rows/core), kept SBUF-resident
(16 MiB) so HBM traffic is one read + one write of the shard.
"""

import sys

sys.path.insert(0, "/opt/trn_rl_repo")

import numpy as np

N_CORES = 8
ROWS, COLS = 8192, 4096
R_CORE = ROWS // N_CORES          # 1024 rows per core
P = 128                           # SBUF partitions
N_TILES = R_CORE // P             # 8 tiles of [128, 4096] per core
CHUNK = 2048                      # phase-2 free-dim chunk
DEGREE = 3

_CACHE = {}


def _expand_cubic(knots: np.ndarray, coeffs: np.ndarray) -> np.ndarray:
    """Expand de Boor at interval m=3 into monomial coeffs [a0, a1, a2, a3] (float64)."""
    t = np.asarray(knots, dtype=np.float64)
    c = np.asarray(coeffs, dtype=np.float64)
    k = DEGREE
    m = k  # reference clips searchsorted result to [k, n-1] with n-1 == k
    pm = np.polynomial.polynomial
    d = [np.array([c[m - k + j]], dtype=np.float64) for j in range(k + 1)]
    for r in range(1, k + 1):
        for j in range(k, r - 1, -1):
            tl = t[m - k + j]
            tr = t[m + j + 1 - r]
            inv = 1.0 / (tr - tl)
            alpha = np.array([-tl * inv, inv])
            one_m = np.array([1.0 + tl * inv, -inv])
            d[j] = pm.polyadd(pm.polymul(one_m, d[j - 1]), pm.polymul(alpha, d[j]))
    a = np.zeros(4, dtype=np.float64)
    a[: len(d[k])] = d[k]
    return a


def _build_program():
    import concourse.bass as bass
    import concourse.tile as tile
    from concourse import bacc, bass_isa, mybir

    dt = mybir.dt.float32
    OP = mybir.AluOpType
    AX = mybir.AxisListType
    AF = mybir.ActivationFunctionType

    nc = bacc.Bacc("TRN2", target_bir_lowering=False, debug=False, num_devices=N_CORES)
    x_ext = nc.declare_dram_parameter("x", [R_CORE, COLS], dt, isOutput=False)
    ac_ext = nc.declare_dram_parameter("ac", [1, 4], dt, isOutput=False)
    y_ext = nc.declare_dram_parameter("y", [R_CORE, COLS], dt, isOutput=True)

    with tile.TileContext(nc) as tc:
        with (
            tc.tile_pool(name="xp", bufs=1) as xp,
            tc.tile_pool(name="wp", bufs=2) as wp,
            tc.tile_pool(name="small", bufs=1) as small,
            tc.tile_pool(name="dram", bufs=1, space="DRAM") as dram,
        ):
            # Warm the collective path (ncfw queue/ring setup) concurrently
            # with the phase-1 loads so the real AllGather is cheap.
            warm_in = dram.tile([1, 2], dt)
            warm_out = dram.tile([1, 2 * N_CORES], dt)
            wz = small.tile([1, 2], dt)
            nc.vector.memset(wz[:], 0.0)
            nc.sync.dma_start(out=warm_in[:], in_=wz[:])
            nc.gpsimd.collective_compute(
                "AllGather", OP.bypass,
                replica_groups=[list(range(N_CORES))],
                ins=[warm_in[:].opt()], outs=[warm_out[:].opt()],
            )

            # ---------------- phase 1: load + local min/max ----------------
            # Tile 0 is loaded in quarters so the first reduce starts as soon
            # as the first 512 KiB lands instead of after the full 2 MiB.
            N_RED = N_TILES + 3
            xts = []
            rmax8 = small.tile([P, N_RED], dt)
            rmin8 = small.tile([P, N_RED], dt)
            x0 = xp.tile([P, COLS], dt, tag="x0")
            xts.append(x0)
            QC = COLS // 4
            for qq in range(4):
                nc.sync.dma_start(out=x0[:, qq * QC:(qq + 1) * QC],
                                  in_=x_ext[0:P, qq * QC:(qq + 1) * QC])
            for qq in range(4):
                xq = x0[:, qq * QC:(qq + 1) * QC]
                nc.vector.tensor_reduce(rmax8[:, qq:qq + 1], xq, axis=AX.X, op=OP.max)
                nc.vector.tensor_reduce(rmin8[:, qq:qq + 1], xq, axis=AX.X, op=OP.min)
            for t in range(1, N_TILES):
                xt = xp.tile([P, COLS], dt, tag=f"x{t}")
                xts.append(xt)
                nc.sync.dma_start(out=xt[:], in_=x_ext[t * P:(t + 1) * P, :])
                nc.vector.tensor_reduce(rmax8[:, t + 3:t + 4], xt[:], axis=AX.X, op=OP.max)
                nc.vector.tensor_reduce(rmin8[:, t + 3:t + 4], xt[:], axis=AX.X, op=OP.min)

            pk = small.tile([P, 2], dt)
            nc.vector.tensor_reduce(pk[:, 0:1], rmax8[:], axis=AX.X, op=OP.max)
            rmn = small.tile([P, 1], dt)
            nc.vector.tensor_reduce(rmn[:], rmin8[:], axis=AX.X, op=OP.min)
            nc.vector.tensor_scalar_mul(pk[:, 1:2], rmn[:], -1.0)

            # cross-partition: every partition gets (local_max, -local_min)
            par = small.tile([P, 2], dt)
            nc.gpsimd.partition_all_reduce(par[:], pk[:], channels=P,
                                           reduce_op=bass_isa.ReduceOp.max)

            # cross-core: AllGather the pair, reduce on the way back in
            cc_in = dram.tile([1, 2], dt)
            cc_out = dram.tile([1, 2 * N_CORES], dt)
            nc.sync.dma_start(out=cc_in[:], in_=par[0:1, 0:2])
            nc.gpsimd.collective_compute(
                "AllGather", OP.bypass,
                replica_groups=[list(range(N_CORES))],
                ins=[cc_in[:].opt()], outs=[cc_out[:].opt()],
            )
            g = small.tile([1, 2, N_CORES], dt)
            nc.sync.dma_start(out=g[:], in_=cc_out[:].rearrange("p (r j) -> p j r", j=2))
            gg = small.tile([1, 2], dt)
            nc.vector.tensor_reduce(gg[:], g[:], axis=AX.X, op=OP.max)
            GG = small.tile([P, 2], dt)
            nc.gpsimd.partition_broadcast(GG[:], gg[:])

            # host constants in: ac = [e2a=a2/a3, e1a=a1/a3, a3, a0]
            ac_sb = small.tile([1, 4], dt)
            nc.sync.dma_start(out=ac_sb[:], in_=ac_ext[:])
            AC = small.tile([P, 4], dt)
            nc.gpsimd.partition_broadcast(AC[:], ac_sb[:])
            e2a, e1a, a3c, a0c = (AC[:, i:i + 1] for i in range(4))

            # ------- device scalars: normalization + composed coefficients -------
            # s = 1/(gmax + gnm + eps); b = gnm*s    (gnm = -gmin)
            # y = P(s*x + b) = ((x + d2)*x + d1)*x*q3 + q0
            #   d2 = (3b + e2a)*d        (d = 1/s)
            #   d1 = ((3b + 2*e2a)*b + e1a)*d^2
            #   q3 = a3*s^3
            #   q0 = a3*(b + e2a)*b^2 + a3*e1a*b + a0
            cf = small.tile([P, 4], dt)
            d2c, d1c, q3c, q0c = (cf[:, i:i + 1] for i in range(4))
            tmp = small.tile([P, 8], dt)
            dd, s_, b_, u, v, w, s2, p_ = (tmp[:, i:i + 1] for i in range(8))

            nc.vector.scalar_tensor_tensor(dd, GG[:, 0:1], 1e-8, GG[:, 1:2],
                                           op0=OP.add, op1=OP.add)      # d = range+eps
            nc.vector.reciprocal(s_, dd)
            nc.vector.tensor_tensor(b_, GG[:, 1:2], s_, op=OP.mult)     # b = gnm*s

            nc.vector.tensor_scalar_mul(u, b_, 3.0)                     # u = 3b
            nc.vector.tensor_tensor(v, u, e2a, op=OP.add)               # v = 3b+e2a
            nc.vector.tensor_tensor(d2c, v, dd, op=OP.mult)             # d2

            nc.vector.scalar_tensor_tensor(w, e2a, 2.0, u, op0=OP.mult, op1=OP.add)  # w = 2e2a+3b
            nc.vector.tensor_tensor(w, w, b_, op=OP.mult)
            nc.vector.tensor_tensor(w, w, e1a, op=OP.add)               # (3b+2e2a)b+e1a
            nc.vector.tensor_tensor(v, dd, dd, op=OP.mult)              # v = d^2
            nc.vector.tensor_tensor(d1c, w, v, op=OP.mult)              # d1

            nc.vector.tensor_tensor(s2, s_, s_, op=OP.mult)
            nc.vector.tensor_tensor(u, s2, s_, op=OP.mult)              # s^3
            nc.vector.tensor_tensor(q3c, u, a3c, op=OP.mult)            # q3

            nc.vector.tensor_tensor(p_, b_, e2a, op=OP.add)             # b+e2a
            nc.vector.tensor_tensor(p_, p_, b_, op=OP.mult)
            nc.vector.tensor_tensor(p_, p_, e1a, op=OP.add)             # (b+e2a)b+e1a
            nc.vector.tensor_tensor(p_, p_, b_, op=OP.mult)
            nc.vector.tensor_tensor(p_, p_, a3c, op=OP.mult)            # a3*(...)
            nc.vector.tensor_tensor(q0c, p_, a0c, op=OP.add)            # q0

            # ---------------- phase 2: evaluate + store ----------------
            n_chunks = COLS // CHUNK
            for t in range(N_TILES):
                xt = xts[t]
                for h in range(n_chunks):
                    xc = xt[:, h * CHUNK:(h + 1) * CHUNK]
                    w1 = wp.tile([P, CHUNK], dt, tag="w1")
                    w2 = wp.tile([P, CHUNK], dt, tag="w2")
                    nc.vector.scalar_tensor_tensor(w1[:], xc, d2c, xc,
                                                   op0=OP.add, op1=OP.mult)
                    nc.vector.scalar_tensor_tensor(w2[:], w1[:], d1c, xc,
                                                   op0=OP.add, op1=OP.mult)
                    nc.scalar.activation(xc, w2[:], AF.Identity,
                                         bias=q0c, scale=q3c)
                nc.sync.dma_start(out=y_ext[t * P:(t + 1) * P, :], in_=xt[:])

    nc.compile()
    return nc


def kernel(x: np.ndarray, knots: np.ndarray, coeffs: np.ndarray) -> np.ndarray:
    from concourse.bass_utils import run_bass_kernel_spmd

    x = np.ascontiguousarray(np.asarray(x, dtype=np.float32))
    assert x.shape == (ROWS, COLS), x.shape

    a = _expand_cubic(knots, coeffs)
    a3 = a[3] if abs(a[3]) > 1e-30 else 1e-30
    ac = np.array([[a[2] / a3, a[1] / a3, a3, a[0]]], dtype=np.float32)

    if "nc" not in _CACHE:
        _CACHE["nc"] = _build_program()
    nc = _CACHE["nc"]

    shards = [x[i * R_CORE:(i + 1) * R_CORE] for i in range(N_CORES)]
    in_maps = [{"x": s, "ac": ac} for s in shards]

    import os
    trace = bool(int(os.environ.get("KERNEL_TRACE", "0")))
    res = run_bass_kernel_spmd(nc, in_maps, core_ids=list(range(N_CORES)),
                               trace=trace)
    if trace and res.exec_time_ns is not None:
        print(f"HW exec time: {res.exec_time_ns} ns")
        _CACHE["last_exec_time_ns"] = res.exec_time_ns
        _CACHE["last_trace"] = res.instructions_and_trace

    out = np.empty((ROWS, COLS), dtype=np.float32)
    for i in range(N_CORES):
        out[i * R_CORE:(i + 1) * R_CORE] = res.results[i]["y"]
    return out


# revision 4
# speedup vs baseline: 1.1196x; 1.0978x over previous
"""Trainium2 Bass kernel for BSplineLayer: y = BSpline(knots, coeffs, k=3)((x - min(x)) / (max(x) - min(x) + 1e-8)).

Because the reference clips the de Boor interval index to [k, n-1] = [3, 3]
(n = len(knots) - k - 1 = 4 basis functions), the whole layer reduces to a
single cubic polynomial P(xn) evaluated everywhere, with coefficients that
depend only on knots/coeffs.  The device computes the global min/max
(all-gathered across the 8 cores), folds the normalization into the cubic,
and evaluates it with two fused scalar_tensor_tensor passes + one ACT pass.

Per-core layout: x is sharded row-wise (1024 rows/core), kept SBUF-resident
(16 MiB) so HBM traffic is one read + one write of the shard.
"""

import sys

sys.path.insert(0, "/opt/trn_rl_repo")

import numpy as np

N_CORES = 8
ROWS, COLS = 8192, 4096
R_CORE = ROWS // N_CORES          # 1024 rows per core
P = 128                           # SBUF partitions
N_TILES = R_CORE // P             # 8 tiles of [128, 4096] per core
CHUNK = 2048                      # phase-2 free-dim chunk
DEGREE = 3

_CACHE = {}


def _expand_cubic(knots: np.ndarray, coeffs: np.ndarray) -> np.ndarray:
    """Expand de Boor at interval m=3 into monomial coeffs [a0, a1, a2, a3] (float64)."""
    t = np.asarray(knots, dtype=np.float64)
    c = np.asarray(coeffs, dtype=np.float64)
    k = DEGREE
    m = k  # reference clips searchsorted result to [k, n-1] with n-1 == k
    pm = np.polynomial.polynomial
    d = [np.array([c[m - k + j]], dtype=np.float64) for j in range(k + 1)]
    for r in range(1, k + 1):
        for j in range(k, r - 1, -1):
            tl = t[m - k + j]
            tr = t[m + j + 1 - r]
            inv = 1.0 / (tr - tl)
            alpha = np.array([-tl * inv, inv])
            one_m = np.array([1.0 + tl * inv, -inv])
            d[j] = pm.polyadd(pm.polymul(one_m, d[j - 1]), pm.polymul(alpha, d[j]))
    a = np.zeros(4, dtype=np.float64)
    a[: len(d[k])] = d[k]
    return a


def _build_program():
    import concourse.bass as bass
    import concourse.tile as tile
    from concourse import bacc, bass_isa, mybir

    dt = mybir.dt.float32
    OP = mybir.AluOpType
    AX = mybir.AxisListType
    AF = mybir.ActivationFunctionType

    nc = bacc.Bacc("TRN2", target_bir_lowering=False, debug=False, num_devices=N_CORES)
    x_ext = nc.declare_dram_parameter("x", [R_CORE, COLS], dt, isOutput=False)
    ac_ext = nc.declare_dram_parameter("ac", [1, 4], dt, isOutput=False)
    y_ext = nc.declare_dram_parameter("y", [R_CORE, COLS], dt, isOutput=True)

    with tile.TileContext(nc) as tc:
        with (
            tc.tile_pool(name="xp", bufs=1) as xp,
            tc.tile_pool(name="wp", bufs=2) as wp,
            tc.tile_pool(name="small", bufs=1) as small,
            tc.tile_pool(name="dram", bufs=1, space="DRAM") as dram,
        ):
            # Warm the collective path (ncfw queue/ring setup + core-skew sync)
            # concurrently with the phase-1 loads so the real AllGather is
            # cheap.  Gathers an uninitialized DRAM word on purpose: zero
            # dependencies means the gpsimd stream enqueues it immediately.
            warm_in = dram.tile([1, 2], dt)
            warm_out = dram.tile([1, 2 * N_CORES], dt)
            nc.gpsimd.collective_compute(
                "AllGather", OP.bypass,
                replica_groups=[list(range(N_CORES))],
                ins=[warm_in[:].opt()], outs=[warm_out[:].opt()],
            )

            # ---------------- phase 1: load + local min/max ----------------
            # Tile 0 is loaded in quarters so the first reduce starts as soon
            # as the first 512 KiB lands instead of after the full 2 MiB.
            N_RED = N_TILES + 3
            xts = []
            rmax8 = small.tile([P, N_RED], dt)
            rmin8 = small.tile([P, N_RED], dt)
            x0 = xp.tile([P, COLS], dt, tag="x0")
            xts.append(x0)
            QC = COLS // 4
            for qq in range(4):
                nc.sync.dma_start(out=x0[:, qq * QC:(qq + 1) * QC],
                                  in_=x_ext[0:P, qq * QC:(qq + 1) * QC])
            for qq in range(4):
                xq = x0[:, qq * QC:(qq + 1) * QC]
                nc.vector.tensor_reduce(rmax8[:, qq:qq + 1], xq, axis=AX.X, op=OP.max)
                nc.vector.tensor_reduce(rmin8[:, qq:qq + 1], xq, axis=AX.X, op=OP.min)
            for t in range(1, N_TILES):
                xt = xp.tile([P, COLS], dt, tag=f"x{t}")
                xts.append(xt)
                nc.sync.dma_start(out=xt[:], in_=x_ext[t * P:(t + 1) * P, :])
                nc.vector.tensor_reduce(rmax8[:, t + 3:t + 4], xt[:], axis=AX.X, op=OP.max)
                nc.vector.tensor_reduce(rmin8[:, t + 3:t + 4], xt[:], axis=AX.X, op=OP.min)

            pk = small.tile([P, 2], dt)
            nc.vector.tensor_reduce(pk[:, 0:1], rmax8[:], axis=AX.X, op=OP.max)
            rmn = small.tile([P, 1], dt)
            nc.vector.tensor_reduce(rmn[:], rmin8[:], axis=AX.X, op=OP.min)
            nc.vector.tensor_scalar_mul(pk[:, 1:2], rmn[:], -1.0)

            # cross-partition: every partition gets (local_max, -local_min)
            par = small.tile([P, 2], dt)
            nc.gpsimd.partition_all_reduce(par[:], pk[:], channels=P,
                                           reduce_op=bass_isa.ReduceOp.max)

            # cross-core: AllGather the pair, reduce on the way back in
            cc_in = dram.tile([1, 2], dt)
            cc_out = dram.tile([1, 2 * N_CORES], dt)
            nc.sync.dma_start(out=cc_in[:], in_=par[0:1, 0:2])
            nc.gpsimd.collective_compute(
                "AllGather", OP.bypass,
                replica_groups=[list(range(N_CORES))],
                ins=[cc_in[:].opt()], outs=[cc_out[:].opt()],
            )
            g = small.tile([1, 2, N_CORES], dt)
            nc.sync.dma_start(out=g[:], in_=cc_out[:].rearrange("p (r j) -> p j r", j=2))
            gg = small.tile([1, 2], dt)
            nc.vector.tensor_reduce(gg[:], g[:], axis=AX.X, op=OP.max)
            GG = small.tile([P, 2], dt)
            nc.gpsimd.partition_broadcast(GG[:], gg[:])

            # host constants in: ac = [e2a=a2/a3, e1a=a1/a3, a3, a0]
            ac_sb = small.tile([1, 4], dt)
            nc.sync.dma_start(out=ac_sb[:], in_=ac_ext[:])
            AC = small.tile([P, 4], dt)
            nc.gpsimd.partition_broadcast(AC[:], ac_sb[:])
            e2a, e1a, a3c, a0c = (AC[:, i:i + 1] for i in range(4))

            # ------- device scalars: normalization + composed coefficients -------
            # s = 1/(gmax + gnm + eps); b = gnm*s    (gnm = -gmin)
            # y = P(s*x + b) = ((x + d2)*x + d1)*x*q3 + q0
            #   d2 = (3b + e2a)*d        (d = 1/s)
            #   d1 = ((3b + 2*e2a)*b + e1a)*d^2
            #   q3 = a3*s^3
            #   q0 = a3*(b + e2a)*b^2 + a3*e1a*b + a0
            cf = small.tile([P, 4], dt)
            d2c, d1c, q3c, q0c = (cf[:, i:i + 1] for i in range(4))
            tmp = small.tile([P, 8], dt)
            dd, s_, b_, u, v, w, s2, p_ = (tmp[:, i:i + 1] for i in range(8))

            nc.vector.scalar_tensor_tensor(dd, GG[:, 0:1], 1e-8, GG[:, 1:2],
                                           op0=OP.add, op1=OP.add)      # d = range+eps
            nc.vector.reciprocal(s_, dd)
            nc.vector.tensor_tensor(b_, GG[:, 1:2], s_, op=OP.mult)     # b = gnm*s

            nc.vector.tensor_scalar_mul(u, b_, 3.0)                     # u = 3b
            nc.vector.tensor_tensor(v, u, e2a, op=OP.add)               # v = 3b+e2a
            nc.vector.tensor_tensor(d2c, v, dd, op=OP.mult)             # d2

            nc.vector.scalar_tensor_tensor(w, e2a, 2.0, u, op0=OP.mult, op1=OP.add)  # w = 2e2a+3b
            nc.vector.tensor_tensor(w, w, b_, op=OP.mult)
            nc.vector.tensor_tensor(w, w, e1a, op=OP.add)               # (3b+2e2a)b+e1a
            nc.vector.tensor_tensor(v, dd, dd, op=OP.mult)              # v = d^2
            nc.vector.tensor_tensor(d1c, w, v, op=OP.mult)              # d1

            nc.vector.tensor_tensor(s2, s_, s_, op=OP.mult)
            nc.vector.tensor_tensor(u, s2, s_, op=OP.mult)              # s^3
            nc.vector.tensor_tensor(q3c, u, a3c, op=OP.mult)            # q3

            nc.vector.tensor_tensor(p_, b_, e2a, op=OP.add)             # b+e2a
            nc.vector.tensor_tensor(p_, p_, b_, op=OP.mult)
            nc.vector.tensor_tensor(p_, p_, e1a, op=OP.add)             # (b+e2a)b+e1a
            nc.vector.tensor_tensor(p_, p_, b_, op=OP.mult)
            nc.vector.tensor_tensor(p_, p_, a3c, op=OP.mult)            # a3*(...)
            nc.vector.tensor_tensor(q0c, p_, a0c, op=OP.add)            # q0

            # ---------------- phase 2: evaluate + store ----------------
            n_chunks = COLS // CHUNK
            for t in range(N_TILES):
                xt = xts[t]
                for h in range(n_chunks):
                    xc = xt[:, h * CHUNK:(h + 1) * CHUNK]
                    w1 = wp.tile([P, CHUNK], dt, tag="w1")
                    w2 = wp.tile([P, CHUNK], dt, tag="w2")
                    nc.vector.scalar_tensor_tensor(w1[:], xc, d2c, xc,
                                                   op0=OP.add, op1=OP.mult)
                    nc.vector.scalar_tensor_tensor(w2[:], w1[:], d1c, xc,
                                                   op0=OP.add, op1=OP.mult)
                    nc.scalar.activation(xc, w2[:], AF.Identity,
                                         bias=q0c, scale=q3c)
                nc.sync.dma_start(out=y_ext[t * P:(t + 1) * P, :], in_=xt[:])

    nc.compile()
    return nc


def kernel(x: np.ndarray, knots: np.ndarray, coeffs: np.ndarray) -> np.ndarray:
    from concourse.bass_utils import run_bass_kernel_spmd

    x = np.ascontiguousarray(np.asarray(x, dtype=np.float32))
    assert x.shape == (ROWS, COLS), x.shape

    a = _expand_cubic(knots, coeffs)
    a3 = a[3] if abs(a[3]) > 1e-30 else 1e-30
    ac = np.array([[a[2] / a3, a[1] / a3, a3, a[0]]], dtype=np.float32)

    if "nc" not in _CACHE:
        _CACHE["nc"] = _build_program()
    nc = _CACHE["nc"]

    shards = [x[i * R_CORE:(i + 1) * R_CORE] for i in range(N_CORES)]
    in_maps = [{"x": s, "ac": ac} for s in shards]

    import os
    trace = bool(int(os.environ.get("KERNEL_TRACE", "0")))
    res = run_bass_kernel_spmd(nc, in_maps, core_ids=list(range(N_CORES)),
                               trace=trace)
    if trace and res.exec_time_ns is not None:
        print(f"HW exec time: {res.exec_time_ns} ns")
        _CACHE["last_exec_time_ns"] = res.exec_time_ns
        _CACHE["last_trace"] = res.instructions_and_trace

    out = np.empty((ROWS, COLS), dtype=np.float32)
    for i in range(N_CORES):
        out[i * R_CORE:(i + 1) * R_CORE] = res.results[i]["y"]
    return out


# revision 7
# speedup vs baseline: 1.1945x; 1.0668x over previous
"""Trainium2 Bass kernel for BSplineLayer: y = BSpline(knots, coeffs, k=3)((x - min(x)) / (max(x) - min(x) + 1e-8)).

Because the reference clips the de Boor interval index to [k, n-1] = [3, 3]
(n = len(knots) - k - 1 = 4 basis functions), the whole layer reduces to a
single cubic polynomial P(xn) evaluated everywhere, with coefficients that
depend only on knots/coeffs.  The device computes the global min/max
(all-gathered across the 8 cores), folds the normalization into the cubic,
and evaluates it with two fused scalar_tensor_tensor passes + one ACT pass.

Per-core layout: x is sharded row-wise (1024 rows/core), kept SBUF-resident
(16 MiB) so HBM traffic is one read + one write of the shard.
"""

import sys

sys.path.insert(0, "/opt/trn_rl_repo")

import numpy as np

N_CORES = 8
ROWS, COLS = 8192, 4096
R_CORE = ROWS // N_CORES          # 1024 rows per core
P = 128                           # SBUF partitions
N_TILES = R_CORE // P             # 8 tiles of [128, 4096] per core
CHUNK = 2048                      # phase-2 free-dim chunk
DEGREE = 3

_CACHE = {}


def _expand_cubic(knots: np.ndarray, coeffs: np.ndarray) -> np.ndarray:
    """Expand de Boor at interval m=3 into monomial coeffs [a0, a1, a2, a3] (float64)."""
    t = np.asarray(knots, dtype=np.float64)
    c = np.asarray(coeffs, dtype=np.float64)
    k = DEGREE
    m = k  # reference clips searchsorted result to [k, n-1] with n-1 == k
    pm = np.polynomial.polynomial
    d = [np.array([c[m - k + j]], dtype=np.float64) for j in range(k + 1)]
    for r in range(1, k + 1):
        for j in range(k, r - 1, -1):
            tl = t[m - k + j]
            tr = t[m + j + 1 - r]
            inv = 1.0 / (tr - tl)
            alpha = np.array([-tl * inv, inv])
            one_m = np.array([1.0 + tl * inv, -inv])
            d[j] = pm.polyadd(pm.polymul(one_m, d[j - 1]), pm.polymul(alpha, d[j]))
    a = np.zeros(4, dtype=np.float64)
    a[: len(d[k])] = d[k]
    return a


def _build_program():
    import concourse.bass as bass
    import concourse.tile as tile
    from concourse import bacc, bass_isa, mybir

    dt = mybir.dt.float32
    OP = mybir.AluOpType
    AX = mybir.AxisListType
    AF = mybir.ActivationFunctionType

    nc = bacc.Bacc("TRN2", target_bir_lowering=False, debug=False, num_devices=N_CORES)
    x_ext = nc.declare_dram_parameter("x", [R_CORE, COLS], dt, isOutput=False)
    ac_ext = nc.declare_dram_parameter("ac", [1, 4], dt, isOutput=False)
    y_ext = nc.declare_dram_parameter("y", [R_CORE, COLS], dt, isOutput=True)

    with tile.TileContext(nc) as tc:
        with (
            tc.tile_pool(name="xp", bufs=1) as xp,
            tc.tile_pool(name="wp", bufs=2) as wp,
            tc.tile_pool(name="small", bufs=1) as small,
            tc.tile_pool(name="dram", bufs=1, space="DRAM") as dram,
        ):
            # Warm the collective path (ncfw queue/ring setup + core-skew sync)
            # concurrently with the phase-1 loads so the real AllGather is
            # cheap.  Gathers an uninitialized DRAM word on purpose: zero
            # dependencies means the gpsimd stream enqueues it immediately.
            warm_in = dram.tile([1, 2], dt)
            warm_out = dram.tile([1, 2 * N_CORES], dt)
            nc.gpsimd.collective_compute(
                "AllGather", OP.bypass,
                replica_groups=[list(range(N_CORES))],
                ins=[warm_in[:].opt()], outs=[warm_out[:].opt()],
            )

            # ---------------- phase 1: load + local min/max ----------------
            # Tile 0 is loaded in quarters so the first reduce starts as soon
            # as the first 512 KiB lands instead of after the full 2 MiB.
            N_RED = N_TILES + 3
            xts = []
            rmax8 = small.tile([P, N_RED], dt)
            rmin8 = small.tile([P, N_RED], dt)
            x0 = xp.tile([P, COLS], dt, tag="x0")
            xts.append(x0)
            QC = COLS // 4
            for qq in range(4):
                nc.sync.dma_start(out=x0[:, qq * QC:(qq + 1) * QC],
                                  in_=x_ext[0:P, qq * QC:(qq + 1) * QC])
            for qq in range(4):
                xq = x0[:, qq * QC:(qq + 1) * QC]
                nc.vector.tensor_reduce(rmax8[:, qq:qq + 1], xq, axis=AX.X, op=OP.max)
                nc.vector.tensor_reduce(rmin8[:, qq:qq + 1], xq, axis=AX.X, op=OP.min)
            for t in range(1, N_TILES):
                xt = xp.tile([P, COLS], dt, tag=f"x{t}")
                xts.append(xt)
                nc.sync.dma_start(out=xt[:], in_=x_ext[t * P:(t + 1) * P, :])
                nc.vector.tensor_reduce(rmax8[:, t + 3:t + 4], xt[:], axis=AX.X, op=OP.max)
                nc.vector.tensor_reduce(rmin8[:, t + 3:t + 4], xt[:], axis=AX.X, op=OP.min)

            pk = small.tile([P, 2], dt)
            nc.vector.tensor_reduce(pk[:, 0:1], rmax8[:], axis=AX.X, op=OP.max)
            rmn = small.tile([P, 1], dt)
            nc.vector.tensor_reduce(rmn[:], rmin8[:], axis=AX.X, op=OP.min)
            nc.vector.tensor_scalar_mul(pk[:, 1:2], rmn[:], -1.0)

            # cross-partition: every partition gets (local_max, -local_min)
            par = small.tile([P, 2], dt)
            nc.gpsimd.partition_all_reduce(par[:], pk[:], channels=P,
                                           reduce_op=bass_isa.ReduceOp.max)

            # cross-core: AllGather the pair, reduce on the way back in
            cc_in = dram.tile([1, 2], dt)
            cc_out = dram.tile([1, 2 * N_CORES], dt)
            nc.sync.dma_start(out=cc_in[:], in_=par[0:1, 0:2])
            nc.gpsimd.collective_compute(
                "AllGather", OP.bypass,
                replica_groups=[list(range(N_CORES))],
                ins=[cc_in[:].opt()], outs=[cc_out[:].opt()],
            )
            g = small.tile([1, 2, N_CORES], dt)
            nc.sync.dma_start(out=g[:], in_=cc_out[:].rearrange("p (r j) -> p j r", j=2))
            gg = small.tile([1, 2], dt)
            nc.vector.tensor_reduce(gg[:], g[:], axis=AX.X, op=OP.max)
            GG = small.tile([P, 2], dt)
            nc.gpsimd.partition_broadcast(GG[:], gg[:])

            # host constants in: ac = [e2a=a2/a3, e1a=a1/a3, a3, a0]
            ac_sb = small.tile([1, 4], dt)
            nc.sync.dma_start(out=ac_sb[:], in_=ac_ext[:])
            AC = small.tile([P, 4], dt)
            nc.gpsimd.partition_broadcast(AC[:], ac_sb[:])
            e2a, e1a, a3c, a0c = (AC[:, i:i + 1] for i in range(4))

            # ------- device scalars: normalization + composed coefficients -------
            # s = 1/(gmax + gnm + eps); b = gnm*s    (gnm = -gmin)
            # y = P(s*x + b) = ((x + d2)*x + d1)*x*q3 + q0
            #   d2 = (3b + e2a)*d        (d = 1/s)
            #   d1 = ((3b + 2*e2a)*b + e1a)*d^2
            #   q3 = a3*s^3
            #   q0 = a3*(b + e2a)*b^2 + a3*e1a*b + a0
            cf = small.tile([P, 8], dt)
            d2c, d1c, q3c, q0c, g1c, g2c, alc = (cf[:, i:i + 1] for i in range(7))
            tmp = small.tile([P, 10], dt)
            dd, s_, b_, u, v, w, s2, p_, de_, _sp = (tmp[:, i:i + 1] for i in range(10))

            nc.vector.scalar_tensor_tensor(dd, GG[:, 0:1], 1e-8, GG[:, 1:2],
                                           op0=OP.add, op1=OP.add)      # d = range+eps
            nc.vector.reciprocal(s_, dd)
            nc.vector.tensor_tensor(b_, GG[:, 1:2], s_, op=OP.mult)     # b = gnm*s

            nc.vector.tensor_scalar_mul(u, b_, 3.0)                     # u = 3b
            nc.vector.tensor_tensor(v, u, e2a, op=OP.add)               # v = 3b+e2a
            nc.vector.tensor_tensor(d2c, v, dd, op=OP.mult)             # d2

            nc.vector.scalar_tensor_tensor(w, e2a, 2.0, u, op0=OP.mult, op1=OP.add)  # w = 2e2a+3b
            nc.vector.tensor_tensor(w, w, b_, op=OP.mult)
            nc.vector.tensor_tensor(w, w, e1a, op=OP.add)               # (3b+2e2a)b+e1a
            nc.vector.tensor_tensor(v, dd, dd, op=OP.mult)              # v = d^2
            nc.vector.tensor_tensor(d1c, w, v, op=OP.mult)              # d1

            nc.vector.tensor_tensor(s2, s_, s_, op=OP.mult)
            nc.vector.tensor_tensor(u, s2, s_, op=OP.mult)              # s^3
            nc.vector.tensor_tensor(q3c, u, a3c, op=OP.mult)            # q3

            nc.vector.tensor_tensor(p_, b_, e2a, op=OP.add)             # b+e2a
            nc.vector.tensor_tensor(g2c, p_, a3c, op=OP.mult)           # g2 = a3*(b+e2a)
            nc.vector.tensor_tensor(p_, p_, b_, op=OP.mult)
            nc.vector.tensor_tensor(p_, p_, e1a, op=OP.add)             # (b+e2a)b+e1a
            nc.vector.tensor_tensor(p_, p_, b_, op=OP.mult)
            nc.vector.tensor_tensor(p_, p_, a3c, op=OP.mult)            # a3*(...)
            nc.vector.tensor_tensor(q0c, p_, a0c, op=OP.add)            # q0

            # form-B extras: t1 = g1*x + g2 ; y = (xn^2 + alpha)*t1 + delta
            nc.vector.tensor_tensor(g1c, a3c, s_, op=OP.mult)           # g1 = a3*s
            nc.vector.tensor_copy(alc, e1a)                             # alpha = a1/a3
            nc.vector.tensor_tensor(de_, e2a, e1a, op=OP.mult)
            nc.vector.tensor_tensor(de_, de_, a3c, op=OP.mult)
            nc.vector.tensor_tensor(de_, a0c, de_, op=OP.subtract)      # delta

            # ACT-owned copies of (s, b, delta): form-B ACT ops then wait on
            # at most one foreign semaphore (the wait-slot limit workaround).
            actsb = small.tile([P, 3], dt)
            nc.scalar.copy(actsb[:, 0:2], tmp[:, 1:3])                  # s, b
            nc.scalar.copy(actsb[:, 2:3], de_)                          # delta

            # ---------------- phase 2: evaluate + store ----------------
            # Mixed forms balance DVE vs ACT:
            #  form A (1 in 4 chunks): w1=(x+d2)x ; w2=(w1+d1)x ; y=q3*w2+q0
            #    -> DVE 2 STT (4.6us), ACT 1 (2.1us)
            #  form B (3 in 4): xsq=Square(s*x+b)=xn^2 ; t1=g1*x+g2 ;
            #    u=(xsq+alpha)*t1 ; y=u+delta
            #    -> DVE TS@2x + STT (3.4us), ACT 2 (4.2us)
            n_chunks = COLS // CHUNK
            ci = 0
            for t in range(N_TILES):
                xt = xts[t]
                for h in range(n_chunks):
                    xc = xt[:, h * CHUNK:(h + 1) * CHUNK]
                    if ci % 4 == 3:
                        w1 = wp.tile([P, CHUNK], dt, tag="w1")
                        w2 = wp.tile([P, CHUNK], dt, tag="w2")
                        nc.vector.scalar_tensor_tensor(w1[:], xc, d2c, xc,
                                                       op0=OP.add, op1=OP.mult)
                        nc.vector.scalar_tensor_tensor(w2[:], w1[:], d1c, xc,
                                                       op0=OP.add, op1=OP.mult)
                        nc.scalar.activation(xc, w2[:], AF.Identity,
                                             bias=q0c, scale=q3c)
                    else:
                        xsq = wp.tile([P, CHUNK], dt, tag="w1")
                        t1 = wp.tile([P, CHUNK], dt, tag="w2")
                        uu = wp.tile([P, CHUNK], dt, tag="u")
                        nc.scalar.activation(xsq[:], xc, AF.Square,
                                             bias=actsb[:, 1:2], scale=actsb[:, 0:1])
                        nc.vector.tensor_scalar(t1[:], xc, g1c, g2c,
                                                op0=OP.mult, op1=OP.add)
                        nc.vector.scalar_tensor_tensor(uu[:], xsq[:], alc, t1[:],
                                                       op0=OP.add, op1=OP.mult)
                        nc.scalar.activation(xc, uu[:], AF.Identity,
                                             bias=actsb[:, 2:3], scale=1.0)
                    ci += 1
                nc.sync.dma_start(out=y_ext[t * P:(t + 1) * P, :], in_=xt[:])

    nc.compile()
    return nc


def kernel(x: np.ndarray, knots: np.ndarray, coeffs: np.ndarray) -> np.ndarray:
    from concourse.bass_utils import run_bass_kernel_spmd

    x = np.ascontiguousarray(np.asarray(x, dtype=np.float32))
    assert x.shape == (ROWS, COLS), x.shape

    a = _expand_cubic(knots, coeffs)
    a3 = a[3] if abs(a[3]) > 1e-30 else 1e-30
    ac = np.array([[a[2] / a3, a[1] / a3, a3, a[0]]], dtype=np.float32)

    if "nc" not in _CACHE:
        _CACHE["nc"] = _build_program()
    nc = _CACHE["nc"]

    shards = [x[i * R_CORE:(i + 1) * R_CORE] for i in range(N_CORES)]
    in_maps = [{"x": s, "ac": ac} for s in shards]

    import os
    trace = bool(int(os.environ.get("KERNEL_TRACE", "0")))
    res = run_bass_kernel_spmd(nc, in_maps, core_ids=list(range(N_CORES)),
                               trace=trace)
    if trace and res.exec_time_ns is not None:
        print(f"HW exec time: {res.exec_time_ns} ns")
        _CACHE["last_exec_time_ns"] = res.exec_time_ns
        _CACHE["last_trace"] = res.instructions_and_trace

    out = np.empty((ROWS, COLS), dtype=np.float32)
    for i in range(N_CORES):
        out[i * R_CORE:(i + 1) * R_CORE] = res.results[i]["y"]
    return out
